# revision 36
# baseline (speedup 1.0000x reference)
"""BitSSM fused kernel for 8 Trainium2 NeuronCores.

Strategy
--------
Data-parallel over tokens: B*S = 16384 tokens split into 8 shards of 2048.
All ops are token-local except the causal depthwise conv (K=4), whose
3-token left halo is precomputed on the host per shard (in_proj of the
3 preceding tokens, or the value that makes x_inner==0 at sequence start).

On-device layout is channel-major [channel_partition, token_free]:
  in_proj  : psum[c,t]  = sum_d WqT_in[d,c] * xT[d,t]      (fp8 x bf16 matmul)
  conv+silu: xc = silu(sum_k wc[c,k]*xi[c,t-3+k] + bc[c])  (DVE taps + ACT)
  x_proj   : gate = sigmoid(s_x * (WqT_x.T @ xc8) + b_x)   (fp8 x fp8 DoubleRow)
  y        : y = xc * gate                                  (DVE)
  out_proj : out = s_out * (WqT_out.T @ y) + b_out          (fp8 x bf16 matmul)

BitNet quantization is done on the host: weights quantize to exactly
{-1,0,1}, which fp8e4m3 represents exactly, so the only precision loss is
the rounding of the moving (activation) operand. The x_proj matmul runs
with BOTH operands fp8e4m3 in DoubleRow mode (2 contraction rows/cycle):
its output delta only feeds a sigmoid (delta std ~0.06), so the fp8
quantization of xc is damped ~16x and costs no accuracy (sim: 2.6e-3 vs
2.36e-3 all-bf16). The y-path keeps a bf16 xc so out_proj sees full
precision. in_proj/out_proj moving operands stay bf16: their outputs feed
the result directly and fp8 there fails the 2e-2 gate (sim: 2.3e-2).
Scales are folded into the conv weights / activation scale immediates.
Only the first D_INNER rows of w_in (x_and_res[..., :D_INNER]) and of
w_x (ssm_params[..., :D_INNER]) are ever used downstream, so the rest is
never computed.
"""

import sys

if '/opt/trn_rl_repo' not in sys.path:
    sys.path.insert(0, '/opt/trn_rl_repo')

import numpy as np
import ml_dtypes

D_MODEL, D_STATE, D_INNER = 1024, 16, 2048
EPS = 1e-5
B, S = 4, 4096
N_CORES = 8
T = (B * S) // N_CORES          # tokens per core
W = 512                         # token tile width
NT = T // W                     # token tiles per core
KI = D_MODEL // 128             # contraction tiles for in_proj
KC = D_INNER // 128             # contraction tiles for x_proj/out_proj
CT = D_INNER // 128             # channel tiles of d_inner
DT = D_MODEL // 128             # channel tiles of d_model

_BUILD_CACHE = {}



def _build(s_x: float, s_out: float):
    import concourse.tile as tile
    from concourse import bacc, mybir


    nc = bacc.Bacc("TRN2", target_bir_lowering=False, debug=False)
    f32 = mybir.dt.float32
    bf16 = mybir.dt.bfloat16
    fp8 = mybir.dt.float8e4
    AF = mybir.ActivationFunctionType
    ALU = mybir.AluOpType

    xT_d = nc.dram_tensor("xT", [D_MODEL, T], bf16, kind="ExternalInput")
    wi_d = nc.dram_tensor("wi", [D_MODEL, D_INNER], fp8, kind="ExternalInput")
    # wx packed pair-major on host: [128, KC*D_INNER], row p col k*D_INNER+c
    # = wxT[k*128+p, c]; device tile [128, KC, D_INNER] for DoubleRow pairs
    wx_d = nc.dram_tensor("wx_pk", [128, KC * D_INNER], fp8, kind="ExternalInput")
    wo_d = nc.dram_tensor("wo", [D_INNER, D_MODEL], fp8, kind="ExternalInput")
    # conv taps (s_in * w_conv) packed [128, CT*4]; fused conv bias [128, CT]
    wc_d = nc.dram_tensor("wc_v8", [128, CT * 4], f32, kind="ExternalInput")
    bc_d = nc.dram_tensor("bc", [128, CT], f32, kind="ExternalInput")
    bx_d = nc.dram_tensor("bx", [128, CT], f32, kind="ExternalInput")
    bo_d = nc.dram_tensor("bo", [128, DT], f32, kind="ExternalInput")
    # 4-token halo (only the last 3 are used by the conv): 4 bf16 = 8 bytes
    # per partition keeps the halo DMAs aligned and fully disjoint from the
    # psum-copy region of the xi tiles
    h0_d = nc.dram_tensor("h0", [128, CT * 4], bf16, kind="ExternalInput")
    out_d = nc.dram_tensor("out", [D_MODEL, T], bf16, kind="ExternalOutput")

    with tile.TileContext(nc) as tc:
        with (
            tc.tile_pool(name="weights", bufs=1) as wpool,
            tc.tile_pool(name="consts", bufs=1) as cpool,
            tc.tile_pool(name="xin", bufs=2) as xpool,
            tc.tile_pool(name="xi", bufs=2) as xipool,
            tc.tile_pool(name="acc", bufs=1) as accpool,
            tc.tile_pool(name="xc", bufs=1) as xcpool,
            tc.tile_pool(name="xc8", bufs=2) as xc8pool,
            tc.tile_pool(name="gate", bufs=3) as gatepool,
            tc.tile_pool(name="y", bufs=1) as ypool,
            tc.tile_pool(name="outp", bufs=1) as opool,
            tc.tile_pool(name="ps_in", bufs=4, space="PSUM") as ps_in,
            tc.tile_pool(name="ps_x", bufs=3, space="PSUM") as ps_x,
            tc.tile_pool(name="ps_o", bufs=1, space="PSUM") as ps_o,
        ):
            # ---- in_proj weights + constants first (gpsimd queue), so the
            # sync queue's first xT tiles aren't stuck behind 8 MB of weights
            wi_t = []
            for k in range(KI):
                t = wpool.tile([128, D_INNER], fp8, tag=f"wi{k}", name=f"wi{k}")
                nc.gpsimd.dma_start(t[:], wi_d[k * 128:(k + 1) * 128, :])
                wi_t.append(t)
            wc_t = cpool.tile([128, CT * 4], f32, name="wc_t")
            nc.gpsimd.dma_start(wc_t[:], wc_d[:, :])
            bc_t = cpool.tile([128, CT], f32, name="bc_t")
            nc.gpsimd.dma_start(bc_t[:], bc_d[:, :])
            bx_t = cpool.tile([128, CT], f32, name="bx_t")
            nc.gpsimd.dma_start(bx_t[:], bx_d[:, :])
            bo_t = cpool.tile([128, DT], f32, name="bo_t")
            nc.gpsimd.dma_start(bo_t[:], bo_d[:, :])

            # ---- PE warmup: ~5us of dummy matmuls during the initial DMA
            # wait so HAM un-throttles (1.2 -> 2.4 GHz) before real work
            warm = cpool.tile([128, 128], bf16, name="warm")
            nc.vector.memset(warm[:], 0.0)
            ps_w = ps_in.tile([128, 128], f32, tag="psin", name="ps_warm")
            for i in range(40):
                nc.tensor.matmul(ps_w[:], warm[:], warm[:],
                                 start=(i == 0), stop=(i == 39))

            # xi tiles hold [4-token halo | W tokens] of raw in_proj output,
            # bf16 so the conv taps run in the DVE 2x mode. The halo columns
            # are written by the PREVIOUS tile's a_group (SBUF->SBUF DMA),
            # or by h0 DMAs for j=0. Column 0 is never read.
            xi_tiles = {}

            def alloc_xi(j):
                if j in xi_tiles or j >= NT:
                    return
                xi_tiles[j] = [
                    xipool.tile([128, 4 + W], bf16, tag=f"xi{ct}",
                                name=f"xi{ct}_{j}", uniquify=True)
                    for ct in range(CT)]

            # h0 halo DMAs ride the scalar queue: it is idle until the first
            # silu (~14us), and this keeps 16 small issues off the gpsimd
            # queue so the wx tiles arrive sooner
            alloc_xi(0)
            for ct in range(CT):
                nc.scalar.dma_start(xi_tiles[0][ct][:, 0:4],
                                    h0_d[:, ct * 4:ct * 4 + 4])

            # first t-tile's activations on the sync queue, in parallel
            xt_tiles = {}
            for j in range(NT):
                if j > 0:
                    continue
                xt_tiles[j] = []
                for k in range(KI):
                    t = xpool.tile([128, W], bf16, tag=f"xt{k}", name=f"xt{k}_{j}")
                    nc.sync.dma_start(
                        t[:], xT_d[k * 128:(k + 1) * 128, j * W:(j + 1) * W])
                    xt_tiles[j].append(t)

            # remaining weights behind the first x tile
            wx_t = wpool.tile([128, KC, D_INNER], fp8, tag="wx", name="wx_t")
            for k in range(KC):
                nc.gpsimd.dma_start(
                    wx_t[:, k, :], wx_d[:, k * D_INNER:(k + 1) * D_INNER])
            wo_t = []
            for k in range(KC):
                t = wpool.tile([128, D_MODEL], fp8, tag=f"wo{k}", name=f"wo{k}")
                nc.gpsimd.dma_start(t[:], wo_d[k * 128:(k + 1) * 128, :])
                wo_t.append(t)

            xc_tiles = {}
            xc8_tiles = {}

            def load_xt(j):
                if j in xt_tiles or j >= NT:
                    return
                xt_tiles[j] = []
                for k in range(KI):
                    t = xpool.tile([128, W], bf16, tag=f"xt{k}", name=f"xt{k}_{j}")
                    nc.sync.dma_start(
                        t[:], xT_d[k * 128:(k + 1) * 128, j * W:(j + 1) * W])
                    xt_tiles[j].append(t)

            def a_group(j, ct):
                """in_proj + conv for one channel tile; silu deferred."""
                xt = xt_tiles[j]
                ps = ps_in.tile([128, W], f32, tag="psin", name=f"psin{ct}_{j}")
                for k in range(KI):
                    nc.tensor.matmul(
                        ps[:], wi_t[k][:, ct * 128:(ct + 1) * 128], xt[k][:],
                        start=(k == 0), stop=(k == KI - 1))
                xi = xi_tiles[j][ct]
                if j == 0:
                    # prologue: ScE is idle, and every us of DVE saved here
                    # pulls the whole xc8(0) chain (which gates B(0)) earlier
                    nc.scalar.activation(xi[:, 4:4 + W], ps[:], AF.Copy)
                else:
                    nc.vector.tensor_copy(xi[:, 4:4 + W], ps[:])
                if j + 1 < NT:
                    # halo for the next token tile: SBUF->SBUF DMA, off DVE;
                    # last 4 tokens = cols [W, W+4)
                    nc.sync.dma_start(xi_tiles[j + 1][ct][:, 0:4],
                                      xi[:, W:W + 4])
                acc0 = accpool.tile([128, W], bf16, tag="accA", bufs=2,
                                    name=f"acc{ct}a_{j}")
                acc1 = accpool.tile([128, W], bf16, tag=f"acc{ct}b",
                                    name=f"acc{ct}b_{j}")
                ve = nc.vector
                ve.tensor_scalar_mul(
                    acc0[:], xi[:, 1:1 + W], wc_t[:, ct * 4:ct * 4 + 1])
                pp = [acc0, acc1]
                for k in range(1, 4):
                    src, dst = pp[(k - 1) % 2], pp[k % 2]
                    ve.scalar_tensor_tensor(
                        dst[:], xi[:, 1 + k:1 + k + W],
                        wc_t[:, ct * 4 + k:ct * 4 + k + 1],
                        src[:], op0=ALU.mult, op1=ALU.add)
                return pp[3 % 2]

            def a_silu(j, ct, acc):
                xct = xcpool.tile([128, W], bf16, tag=f"xc{ct}",
                                  name=f"xc{ct}_{j}")
                si = nc.scalar.activation(xct[:], acc[:], AF.Silu,
                                          bias=bc_t[:, ct:ct + 1], scale=1.0)
                si8 = nc.scalar.activation(xc8_tiles[j][:, ct, :], acc[:],
                                           AF.Silu, bias=bc_t[:, ct:ct + 1],
                                           scale=1.0)
                xc_tiles[j].append(xct)
                return si, si8

            def b_xproj_group(j, c2, y):
                xc = xc_tiles[j]
                xc8 = xc8_tiles[j]
                ps = ps_x.tile([128, W], f32, tag="psx", name=f"psx{c2}_{j}")
                for i in range(KC // 2):
                    nc.tensor.matmul(
                        ps[:], wx_t[:, 2 * i:2 * i + 2, c2 * 128:(c2 + 1) * 128],
                        xc8[:, 2 * i:2 * i + 2, :],
                        start=(i == 0), stop=(i == KC // 2 - 1),
                        perf_mode=mybir.MatmulPerfMode.DoubleRow)
                gate = gatepool.tile([128, W], bf16, tag="gate", name=f"g{c2}_{j}")
                sg = nc.scalar.activation(gate[:], ps[:], AF.Sigmoid,
                                          bias=bx_t[:, c2:c2 + 1], scale=s_x)
                sig_insts.append(sg)
                yt = ypool.tile([128, W], bf16, tag=f"y{c2}", name=f"y{c2}_{j}")
                nc.gpsimd.tensor_mul(yt[:], xc[c2][:], gate[:])
                y.append(yt)

            def b_outproj_group(j, dt, y):
                ps = ps_o.tile([128, W], f32, tag="pso", name=f"pso{dt}_{j}")
                for k in range(KC):
                    nc.tensor.matmul(
                        ps[:], wo_t[k][:, dt * 128:(dt + 1) * 128], y[k][:],
                        start=(k == 0), stop=(k == KC - 1))
                ot = opool.tile([128, W], bf16, tag=f"ot{dt}", name=f"ot{dt}_{j}")
                # identity is in every ACT table set: no table-switch cost,
                # and ScE has the fast PSUM port
                nc.scalar.activation(ot[:], ps[:], AF.Identity,
                                     bias=bo_t[:, dt:dt + 1], scale=s_out)
                # alternate queues so the tail's 8 DMAs don't serialize
                q = nc.sync if dt % 2 == 0 else nc.scalar
                q.dma_start(
                    out_d[dt * 128:(dt + 1) * 128, j * W:(j + 1) * W], ot[:])

            def emit_a_phase(j):
                """Emit all of in_proj+conv+silu for tile j (pipeline fill).
                Silus go inline (no sigmoids yet, so no table thrash): each
                xc8 slice is ready ~1 conv-group after its acc, letting the
                first B-phase x_proj groups start much earlier."""
                xc_tiles[j] = []
                xc8_tiles[j] = xc8pool.tile([128, CT, W], fp8, tag="xc8",
                                            name=f"xc8_{j}")
                for ct in range(CT):
                    a_silu(j, ct, a_group(j, ct))

            # ---- software pipeline: B(j) interleaved with A(j+1) ----
            from concourse.tile import add_dep_helper
            sig_insts = []
            load_xt(0)
            load_xt(1)
            alloc_xi(1)
            emit_a_phase(0)
            for j in range(NT):
                nxt = j + 1
                load_xt(nxt + 1)
                alloc_xi(nxt + 1)
                if nxt < NT:
                    xc_tiles[nxt] = []
                    xc8_tiles[nxt] = xc8pool.tile([128, CT, W], fp8, tag="xc8",
                                                  name=f"xc8_{nxt}")
                a_accs = []
                y = []
                # 16 x_proj groups interleaved with 16 A-groups of next tile
                for c2 in range(CT):
                    b_xproj_group(j, c2, y)
                    if nxt < NT:
                        a_accs.append((c2, a_group(nxt, c2)))
                for dt in range(DT):
                    b_outproj_group(j, dt, y)
                if nxt < NT:
                    prev_act = sig_insts[-1] if sig_insts else None
                    for ct, acc in a_accs:
                        for si in a_silu(nxt, ct, acc):
                            if prev_act is not None:
                                add_dep_helper(si.ins, prev_act.ins, sync=False,
                                               reason="group ACT funcs to avoid table thrash")
                            prev_act = si

    nc.compile()
    return nc


def _quantize(w):
    s = np.float32(max(np.abs(w).mean(dtype=np.float64), EPS))
    return np.clip(np.round(w / s), -1.0, 1.0).astype(np.float32), s


def kernel(x, w_in, b_in, w_conv, b_conv, w_x, b_x, w_out, b_out,
           _trace=False, _trace_kwargs=None):
    from concourse import bass_utils

    x = np.asarray(x, dtype=np.float32)
    w_in = np.asarray(w_in, dtype=np.float32)
    b_in = np.asarray(b_in, dtype=np.float32)
    w_conv = np.asarray(w_conv, dtype=np.float32)
    b_conv = np.asarray(b_conv, dtype=np.float32)
    w_x = np.asarray(w_x, dtype=np.float32)
    b_x = np.asarray(b_x, dtype=np.float32)
    w_out = np.asarray(w_out, dtype=np.float32)
    b_out = np.asarray(b_out, dtype=np.float32)

    # ---- host-side BitNet quantization (exact ternary) ----
    wq_in, s_in = _quantize(w_in)     # [2*D_INNER, D_MODEL]
    wq_x, s_x = _quantize(w_x)        # [D_STATE+D_MODEL+D_INNER, D_INNER]
    wq_out, s_out = _quantize(w_out)  # [D_MODEL, D_INNER]
    wq_in = wq_in[:D_INNER]           # res half unused downstream
    wq_x_d = wq_x[:D_INNER]           # only delta rows used

    fp8 = ml_dtypes.float8_e4m3
    wiT = np.ascontiguousarray(wq_in.T).astype(fp8)       # [D_MODEL, D_INNER]
    wxT = np.ascontiguousarray(wq_x_d.T).astype(fp8)      # [D_INNER, D_INNER]
    woT = np.ascontiguousarray(wq_out.T).astype(fp8)      # [D_INNER, D_MODEL]
    # pair-major for DoubleRow: [128, KC*D_INNER], row p col k*D_INNER+c
    wx_pk = np.ascontiguousarray(
        wxT.reshape(KC, 128, D_INNER).transpose(1, 0, 2)
        .reshape(128, KC * D_INNER))

    # conv taps with in_proj scale folded in; bias absorbs b_in through the taps
    wc = (s_in * w_conv[:, 0, :]).astype(np.float32)             # [D_INNER, 4]
    bc = (b_in[:D_INNER] * w_conv[:, 0, :].sum(axis=1)
          + b_conv).astype(np.float32)                           # [D_INNER]
    bx = b_x[:D_INNER].astype(np.float32)
    bo = b_out.astype(np.float32)

    # pack per-channel constants as [128, CT] (partition = channel % 128)
    def pack(v, ncols):
        return np.ascontiguousarray(
            v.reshape(ncols, 128).T.astype(np.float32))  # [128, ncols]

    wc_pk = np.ascontiguousarray(
        wc.reshape(CT, 128, 4).transpose(1, 0, 2).reshape(128, CT * 4))
    bc_pk = pack(bc, CT)
    bx_pk = pack(bx, CT)
    bo_pk = pack(bo, DT)

    # ---- shard inputs ----
    x_flat = x.reshape(B * S, D_MODEL)
    xT = np.ascontiguousarray(x_flat.T)                   # [D_MODEL, B*S] f32
    xT_bf = xT.astype(ml_dtypes.bfloat16)

    # raw in_proj value that makes x_inner == 0 (sequence-start padding)
    pad_raw = (-b_in[:D_INNER] / s_in).astype(np.float32)

    in_maps = []
    for c in range(N_CORES):
        t0 = c * T
        if t0 % S == 0:
            h0 = np.repeat(pad_raw[:, None], 4, axis=1)   # [D_INNER, 4]
        else:
            h0 = wq_in @ x_flat[t0 - 4:t0].T              # [D_INNER, 4]
        h0_pk = np.ascontiguousarray(
            h0.reshape(CT, 128, 4).transpose(1, 0, 2).reshape(128, CT * 4)
        ).astype(ml_dtypes.bfloat16)
        in_maps.append({
            "xT": np.ascontiguousarray(xT_bf[:, t0:t0 + T]),
            "wi": wiT, "wx_pk": wx_pk, "wo": woT,
            "wc_v8": wc_pk, "bc": bc_pk, "bx": bx_pk, "bo": bo_pk,
            "h0": h0_pk,
        })

    key = (float(s_x), float(s_out))
    if key not in _BUILD_CACHE:
        _BUILD_CACHE[key] = _build(float(s_x), float(s_out))
    nc = _BUILD_CACHE[key]

    kwargs = {}
    if _trace:
        kwargs["trace"] = True
        if _trace_kwargs:
            kwargs.update(_trace_kwargs)
    res = bass_utils.run_bass_kernel_spmd(
        nc, in_maps, core_ids=list(range(N_CORES)), **kwargs)
    kernel.last_results = res

    full = np.concatenate([res.results[c]["out"] for c in range(N_CORES)],
                          axis=1)                          # [D_MODEL, B*S]
    return np.ascontiguousarray(full.T).reshape(B, S, D_MODEL).astype(np.float32)



# revision 41
# speedup vs baseline: 1.1042x; 1.1042x over previous
"""BitSSM fused kernel for 8 Trainium2 NeuronCores.

Strategy
--------
Data-parallel over tokens: B*S = 16384 tokens split into 8 shards of 2048.
All ops are token-local except the causal depthwise conv (K=4), whose
3-token left halo is precomputed on the host per shard (in_proj of the
3 preceding tokens, or the value that makes x_inner==0 at sequence start).

On-device layout is channel-major [channel_partition, token_free]:
  in_proj  : psum[c,t]  = sum_d WqT_in[d,c] * xT[d,t]      (fp8 x bf16 matmul)
  conv+silu: xc = silu(sum_k wc[c,k]*xi[c,t-3+k] + bc[c])  (DVE taps + ACT)
  x_proj   : gate = sigmoid(s_x * (WqT_x.T @ xc8) + b_x)   (fp8 x fp8 DoubleRow)
  y        : y = xc * gate                                  (DVE)
  out_proj : out = s_out * (WqT_out.T @ y) + b_out          (fp8 x bf16 matmul)

BitNet quantization is done on the host: weights quantize to exactly
{-1,0,1}, which fp8e4m3 represents exactly, so the only precision loss is
the rounding of the moving (activation) operand. The x_proj matmul runs
with BOTH operands fp8e4m3 in DoubleRow mode (2 contraction rows/cycle):
its output delta only feeds a sigmoid (delta std ~0.06), so the fp8
quantization of xc is damped ~16x and costs no accuracy (sim: 2.6e-3 vs
2.36e-3 all-bf16). The y-path keeps a bf16 xc so out_proj sees full
precision. in_proj/out_proj moving operands stay bf16: their outputs feed
the result directly and fp8 there fails the 2e-2 gate (sim: 2.3e-2).
Scales are folded into the conv weights / activation scale immediates.
Only the first D_INNER rows of w_in (x_and_res[..., :D_INNER]) and of
w_x (ssm_params[..., :D_INNER]) are ever used downstream, so the rest is
never computed.
"""

import sys

if '/opt/trn_rl_repo' not in sys.path:
    sys.path.insert(0, '/opt/trn_rl_repo')

import numpy as np
import ml_dtypes

D_MODEL, D_STATE, D_INNER = 1024, 16, 2048
EPS = 1e-5
B, S = 4, 4096
N_CORES = 8
T = (B * S) // N_CORES          # tokens per core
W = 512                         # token tile width
NT = T // W                     # token tiles per core
KI = D_MODEL // 128             # contraction tiles for in_proj
KC = D_INNER // 128             # contraction tiles for x_proj/out_proj
CT = D_INNER // 128             # channel tiles of d_inner
DT = D_MODEL // 128             # channel tiles of d_model

_BUILD_CACHE = {}



def _build(s_x: float, s_out: float):
    import concourse.tile as tile
    from concourse import bacc, mybir


    nc = bacc.Bacc("TRN2", target_bir_lowering=False, debug=False)
    f32 = mybir.dt.float32
    bf16 = mybir.dt.bfloat16
    fp8 = mybir.dt.float8e4
    AF = mybir.ActivationFunctionType
    ALU = mybir.AluOpType

    xT_d = nc.dram_tensor("xT", [D_MODEL, T], bf16, kind="ExternalInput")
    wi_d = nc.dram_tensor("wi", [D_MODEL, D_INNER], fp8, kind="ExternalInput")
    # wx packed pair-major on host: [128, KC*D_INNER], row p col k*D_INNER+c
    # = wxT[k*128+p, c]; device tile [128, KC, D_INNER] for DoubleRow pairs
    wx_d = nc.dram_tensor("wx_pk", [128, KC * D_INNER], fp8, kind="ExternalInput")
    wo_d = nc.dram_tensor("wo", [D_INNER, D_MODEL], fp8, kind="ExternalInput")
    # conv taps (s_in * w_conv) packed [128, CT*4]; fused conv bias [128, CT]
    wc_d = nc.dram_tensor("wc_v8", [128, CT * 4], f32, kind="ExternalInput")
    bc_d = nc.dram_tensor("bc", [128, CT], f32, kind="ExternalInput")
    bx_d = nc.dram_tensor("bx", [128, CT], f32, kind="ExternalInput")
    bo_d = nc.dram_tensor("bo", [128, DT], f32, kind="ExternalInput")
    # 4-token halo (only the last 3 are used by the conv): 4 bf16 = 8 bytes
    # per partition keeps the halo DMAs aligned and fully disjoint from the
    # psum-copy region of the xi tiles
    h0_d = nc.dram_tensor("h0", [128, CT * 4], bf16, kind="ExternalInput")
    out_d = nc.dram_tensor("out", [D_MODEL, T], bf16, kind="ExternalOutput")

    with tile.TileContext(nc) as tc:
        with (
            tc.tile_pool(name="weights", bufs=1) as wpool,
            tc.tile_pool(name="consts", bufs=1) as cpool,
            tc.tile_pool(name="xin", bufs=2) as xpool,
            tc.tile_pool(name="xi", bufs=2) as xipool,
            tc.tile_pool(name="acc", bufs=1) as accpool,
            tc.tile_pool(name="xc", bufs=1) as xcpool,
            tc.tile_pool(name="xc8", bufs=2) as xc8pool,
            tc.tile_pool(name="gate", bufs=3) as gatepool,
            tc.tile_pool(name="y", bufs=1) as ypool,
            tc.tile_pool(name="outp", bufs=1) as opool,
            tc.tile_pool(name="ps_in", bufs=3, space="PSUM") as ps_in,
            tc.tile_pool(name="ps_x", bufs=3, space="PSUM") as ps_x,
            tc.tile_pool(name="ps_o", bufs=2, space="PSUM") as ps_o,
        ):
            # ---- in_proj weights + constants first (gpsimd queue), so the
            # sync queue's first xT tiles aren't stuck behind 8 MB of weights
            wi_t = []
            for k in range(KI):
                t = wpool.tile([128, D_INNER], fp8, tag=f"wi{k}", name=f"wi{k}")
                nc.gpsimd.dma_start(t[:], wi_d[k * 128:(k + 1) * 128, :])
                wi_t.append(t)
            wc_t = cpool.tile([128, CT * 4], f32, name="wc_t")
            nc.gpsimd.dma_start(wc_t[:], wc_d[:, :])
            bc_t = cpool.tile([128, CT], f32, name="bc_t")
            nc.gpsimd.dma_start(bc_t[:], bc_d[:, :])
            bx_t = cpool.tile([128, CT], f32, name="bx_t")
            nc.gpsimd.dma_start(bx_t[:], bx_d[:, :])
            bo_t = cpool.tile([128, DT], f32, name="bo_t")
            nc.gpsimd.dma_start(bo_t[:], bo_d[:, :])

            # ---- PE warmup: ~5us of dummy matmuls during the initial DMA
            # wait so HAM un-throttles (1.2 -> 2.4 GHz) before real work
            warm = cpool.tile([128, 128], bf16, name="warm")
            nc.vector.memset(warm[:], 0.0)
            ps_w = ps_in.tile([128, 128], f32, tag="psin", name="ps_warm")
            for i in range(40):
                nc.tensor.matmul(ps_w[:], warm[:], warm[:],
                                 start=(i == 0), stop=(i == 39))

            # xi tiles hold [4-token halo | W tokens] of raw in_proj output,
            # bf16 so the conv taps run in the DVE 2x mode. The halo columns
            # are written by the PREVIOUS tile's a_group (SBUF->SBUF DMA),
            # or by h0 DMAs for j=0. Column 0 is never read.
            xi_tiles = {}

            def alloc_xi(j):
                if j in xi_tiles or j >= NT:
                    return
                xi_tiles[j] = [
                    xipool.tile([128, 4 + W], bf16, tag=f"xi{ct}",
                                name=f"xi{ct}_{j}", uniquify=True)
                    for ct in range(CT)]

            alloc_xi(0)
            for ct in range(CT):
                nc.gpsimd.dma_start(xi_tiles[0][ct][:, 0:4],
                                    h0_d[:, ct * 4:ct * 4 + 4])

            # first t-tile's activations on the sync queue, in parallel
            xt_tiles = {}
            for j in range(NT):
                if j > 0:
                    continue
                xt_tiles[j] = []
                for k in range(KI):
                    t = xpool.tile([128, W], bf16, tag=f"xt{k}", name=f"xt{k}_{j}")
                    nc.sync.dma_start(
                        t[:], xT_d[k * 128:(k + 1) * 128, j * W:(j + 1) * W])
                    xt_tiles[j].append(t)

            # remaining weights behind the first x tile
            wx_t = wpool.tile([128, KC, D_INNER], fp8, tag="wx", name="wx_t")
            for k in range(KC):
                nc.gpsimd.dma_start(
                    wx_t[:, k, :], wx_d[:, k * D_INNER:(k + 1) * D_INNER])
            wo_t = []
            for k in range(KC):
                t = wpool.tile([128, D_MODEL], fp8, tag=f"wo{k}", name=f"wo{k}")
                nc.gpsimd.dma_start(t[:], wo_d[k * 128:(k + 1) * 128, :])
                wo_t.append(t)

            xc_tiles = {}
            xc8_tiles = {}

            def load_xt(j):
                if j in xt_tiles or j >= NT:
                    return
                xt_tiles[j] = []
                for k in range(KI):
                    t = xpool.tile([128, W], bf16, tag=f"xt{k}", name=f"xt{k}_{j}")
                    nc.sync.dma_start(
                        t[:], xT_d[k * 128:(k + 1) * 128, j * W:(j + 1) * W])
                    xt_tiles[j].append(t)

            def a_group(j, ct):
                """in_proj + conv for one channel tile; silu deferred."""
                xt = xt_tiles[j]
                ps = ps_in.tile([128, W], f32, tag="psin", name=f"psin{ct}_{j}")
                for k in range(KI):
                    nc.tensor.matmul(
                        ps[:], wi_t[k][:, ct * 128:(ct + 1) * 128], xt[k][:],
                        start=(k == 0), stop=(k == KI - 1))
                xi = xi_tiles[j][ct]
                nc.vector.tensor_copy(xi[:, 4:4 + W], ps[:])
                if j + 1 < NT:
                    # halo for the next token tile: SBUF->SBUF DMA, off DVE;
                    # last 4 tokens = cols [W, W+4)
                    nc.sync.dma_start(xi_tiles[j + 1][ct][:, 0:4],
                                      xi[:, W:W + 4])
                acc0 = accpool.tile([128, W], bf16, tag="accA", bufs=2,
                                    name=f"acc{ct}a_{j}")
                acc1 = accpool.tile([128, W], bf16, tag=f"acc{ct}b",
                                    name=f"acc{ct}b_{j}")
                ve = nc.vector
                ve.tensor_scalar_mul(
                    acc0[:], xi[:, 1:1 + W], wc_t[:, ct * 4:ct * 4 + 1])
                pp = [acc0, acc1]
                for k in range(1, 4):
                    src, dst = pp[(k - 1) % 2], pp[k % 2]
                    ve.scalar_tensor_tensor(
                        dst[:], xi[:, 1 + k:1 + k + W],
                        wc_t[:, ct * 4 + k:ct * 4 + k + 1],
                        src[:], op0=ALU.mult, op1=ALU.add)
                return pp[3 % 2]

            def a_silu(j, ct, acc):
                xct = xcpool.tile([128, W], bf16, tag=f"xc{ct}",
                                  name=f"xc{ct}_{j}")
                si = nc.scalar.activation(xct[:], acc[:], AF.Silu,
                                          bias=bc_t[:, ct:ct + 1], scale=1.0)
                si8 = nc.scalar.activation(xc8_tiles[j][:, ct, :], acc[:],
                                           AF.Silu, bias=bc_t[:, ct:ct + 1],
                                           scale=1.0)
                xc_tiles[j].append(xct)
                return si, si8

            def b_xproj_group(j, c2, y):
                xc = xc_tiles[j]
                xc8 = xc8_tiles[j]
                ps = ps_x.tile([128, W], f32, tag="psx", name=f"psx{c2}_{j}")
                for i in range(KC // 2):
                    nc.tensor.matmul(
                        ps[:], wx_t[:, 2 * i:2 * i + 2, c2 * 128:(c2 + 1) * 128],
                        xc8[:, 2 * i:2 * i + 2, :],
                        start=(i == 0), stop=(i == KC // 2 - 1),
                        perf_mode=mybir.MatmulPerfMode.DoubleRow)
                gate = gatepool.tile([128, W], bf16, tag="gate", name=f"g{c2}_{j}")
                sg = nc.scalar.activation(gate[:], ps[:], AF.Sigmoid,
                                          bias=bx_t[:, c2:c2 + 1], scale=s_x)
                sig_insts.append(sg)
                yt = ypool.tile([128, W], bf16, tag=f"y{c2}", name=f"y{c2}_{j}")
                nc.gpsimd.tensor_mul(yt[:], xc[c2][:], gate[:])
                y.append(yt)

            def b_outproj_group(j, dt, y):
                ps = ps_o.tile([128, W], f32, tag="pso", name=f"pso{dt}_{j}")
                for k in range(KC):
                    nc.tensor.matmul(
                        ps[:], wo_t[k][:, dt * 128:(dt + 1) * 128], y[k][:],
                        start=(k == 0), stop=(k == KC - 1))
                ot = opool.tile([128, W], bf16, tag=f"ot{dt}", name=f"ot{dt}_{j}")
                # identity is in every ACT table set: no table-switch cost,
                # and ScE has the fast PSUM port
                nc.scalar.activation(ot[:], ps[:], AF.Identity,
                                     bias=bo_t[:, dt:dt + 1], scale=s_out)
                nc.sync.dma_start(
                    out_d[dt * 128:(dt + 1) * 128, j * W:(j + 1) * W], ot[:])

            def emit_a_phase(j):
                """Emit all of in_proj+conv+silu for tile j (pipeline fill)."""
                xc_tiles[j] = []
                xc8_tiles[j] = xc8pool.tile([128, CT, W], fp8, tag="xc8",
                                            name=f"xc8_{j}")
                accs = [a_group(j, ct) for ct in range(CT)]
                for ct in range(CT):
                    a_silu(j, ct, accs[ct])

            # ---- software pipeline: B(j) interleaved with A(j+1) ----
            from concourse.tile import add_dep_helper
            sig_insts = []
            load_xt(0)
            load_xt(1)
            alloc_xi(1)
            emit_a_phase(0)
            for j in range(NT):
                nxt = j + 1
                load_xt(nxt + 1)
                alloc_xi(nxt + 1)
                if nxt < NT:
                    xc_tiles[nxt] = []
                    xc8_tiles[nxt] = xc8pool.tile([128, CT, W], fp8, tag="xc8",
                                                  name=f"xc8_{nxt}")
                a_accs = []
                y = []
                # 16 x_proj groups interleaved with 16 A-groups of next tile
                for c2 in range(CT):
                    b_xproj_group(j, c2, y)
                    if nxt < NT:
                        a_accs.append((c2, a_group(nxt, c2)))
                for dt in range(DT):
                    b_outproj_group(j, dt, y)
                if nxt < NT:
                    prev_act = sig_insts[-1] if sig_insts else None
                    for ct, acc in a_accs:
                        for si in a_silu(nxt, ct, acc):
                            if prev_act is not None:
                                add_dep_helper(si.ins, prev_act.ins, sync=False,
                                               reason="group ACT funcs to avoid table thrash")
                            prev_act = si

    nc.compile()
    return nc


def _quantize(w):
    s = np.float32(max(np.abs(w).mean(dtype=np.float64), EPS))
    return np.clip(np.round(w / s), -1.0, 1.0).astype(np.float32), s


def kernel(x, w_in, b_in, w_conv, b_conv, w_x, b_x, w_out, b_out,
           _trace=False, _trace_kwargs=None):
    from concourse import bass_utils

    x = np.asarray(x, dtype=np.float32)
    w_in = np.asarray(w_in, dtype=np.float32)
    b_in = np.asarray(b_in, dtype=np.float32)
    w_conv = np.asarray(w_conv, dtype=np.float32)
    b_conv = np.asarray(b_conv, dtype=np.float32)
    w_x = np.asarray(w_x, dtype=np.float32)
    b_x = np.asarray(b_x, dtype=np.float32)
    w_out = np.asarray(w_out, dtype=np.float32)
    b_out = np.asarray(b_out, dtype=np.float32)

    # ---- host-side BitNet quantization (exact ternary) ----
    wq_in, s_in = _quantize(w_in)     # [2*D_INNER, D_MODEL]
    wq_x, s_x = _quantize(w_x)        # [D_STATE+D_MODEL+D_INNER, D_INNER]
    wq_out, s_out = _quantize(w_out)  # [D_MODEL, D_INNER]
    wq_in = wq_in[:D_INNER]           # res half unused downstream
    wq_x_d = wq_x[:D_INNER]           # only delta rows used

    fp8 = ml_dtypes.float8_e4m3
    wiT = np.ascontiguousarray(wq_in.T).astype(fp8)       # [D_MODEL, D_INNER]
    wxT = np.ascontiguousarray(wq_x_d.T).astype(fp8)      # [D_INNER, D_INNER]
    woT = np.ascontiguousarray(wq_out.T).astype(fp8)      # [D_INNER, D_MODEL]
    # pair-major for DoubleRow: [128, KC*D_INNER], row p col k*D_INNER+c
    wx_pk = np.ascontiguousarray(
        wxT.reshape(KC, 128, D_INNER).transpose(1, 0, 2)
        .reshape(128, KC * D_INNER))

    # conv taps with in_proj scale folded in; bias absorbs b_in through the taps
    wc = (s_in * w_conv[:, 0, :]).astype(np.float32)             # [D_INNER, 4]
    bc = (b_in[:D_INNER] * w_conv[:, 0, :].sum(axis=1)
          + b_conv).astype(np.float32)                           # [D_INNER]
    bx = b_x[:D_INNER].astype(np.float32)
    bo = b_out.astype(np.float32)

    # pack per-channel constants as [128, CT] (partition = channel % 128)
    def pack(v, ncols):
        return np.ascontiguousarray(
            v.reshape(ncols, 128).T.astype(np.float32))  # [128, ncols]

    wc_pk = np.ascontiguousarray(
        wc.reshape(CT, 128, 4).transpose(1, 0, 2).reshape(128, CT * 4))
    bc_pk = pack(bc, CT)
    bx_pk = pack(bx, CT)
    bo_pk = pack(bo, DT)

    # ---- shard inputs ----
    x_flat = x.reshape(B * S, D_MODEL)
    xT = np.ascontiguousarray(x_flat.T)                   # [D_MODEL, B*S] f32
    xT_bf = xT.astype(ml_dtypes.bfloat16)

    # raw in_proj value that makes x_inner == 0 (sequence-start padding)
    pad_raw = (-b_in[:D_INNER] / s_in).astype(np.float32)

    in_maps = []
    for c in range(N_CORES):
        t0 = c * T
        if t0 % S == 0:
            h0 = np.repeat(pad_raw[:, None], 4, axis=1)   # [D_INNER, 4]
        else:
            h0 = wq_in @ x_flat[t0 - 4:t0].T              # [D_INNER, 4]
        h0_pk = np.ascontiguousarray(
            h0.reshape(CT, 128, 4).transpose(1, 0, 2).reshape(128, CT * 4)
        ).astype(ml_dtypes.bfloat16)
        in_maps.append({
            "xT": np.ascontiguousarray(xT_bf[:, t0:t0 + T]),
            "wi": wiT, "wx_pk": wx_pk, "wo": woT,
            "wc_v8": wc_pk, "bc": bc_pk, "bx": bx_pk, "bo": bo_pk,
            "h0": h0_pk,
        })

    key = (float(s_x), float(s_out))
    if key not in _BUILD_CACHE:
        _BUILD_CACHE[key] = _build(float(s_x), float(s_out))
    nc = _BUILD_CACHE[key]

    kwargs = {}
    if _trace:
        kwargs["trace"] = True
        if _trace_kwargs:
            kwargs.update(_trace_kwargs)
    res = bass_utils.run_bass_kernel_spmd(
        nc, in_maps, core_ids=list(range(N_CORES)), **kwargs)
    kernel.last_results = res

    full = np.concatenate([res.results[c]["out"] for c in range(N_CORES)],
                          axis=1)                          # [D_MODEL, B*S]
    return np.ascontiguousarray(full.T).reshape(B, S, D_MODEL).astype(np.float32)



# revision 42
# speedup vs baseline: 1.1251x; 1.0189x over previous
"""BitSSM fused kernel for 8 Trainium2 NeuronCores.

Strategy
--------
Data-parallel over tokens: B*S = 16384 tokens split into 8 shards of 2048.
All ops are token-local except the causal depthwise conv (K=4), whose
3-token left halo is precomputed on the host per shard (in_proj of the
3 preceding tokens, or the value that makes x_inner==0 at sequence start).

On-device layout is channel-major [channel_partition, token_free]:
  in_proj  : psum[c,t]  = sum_d WqT_in[d,c] * xT[d,t]      (fp8 x bf16 matmul)
  conv+silu: xc = silu(sum_k wc[c,k]*xi[c,t-3+k] + bc[c])  (DVE taps + ACT)
  x_proj   : gate = sigmoid(s_x * (WqT_x.T @ xc8) + b_x)   (fp8 x fp8 DoubleRow)
  y        : y = xc * gate                                  (DVE)
  out_proj : out = s_out * (WqT_out.T @ y) + b_out          (fp8 x bf16 matmul)

BitNet quantization is done on the host: weights quantize to exactly
{-1,0,1}, which fp8e4m3 represents exactly, so the only precision loss is
the rounding of the moving (activation) operand. The x_proj matmul runs
with BOTH operands fp8e4m3 in DoubleRow mode (2 contraction rows/cycle):
its output delta only feeds a sigmoid (delta std ~0.06), so the fp8
quantization of xc is damped ~16x and costs no accuracy (sim: 2.6e-3 vs
2.36e-3 all-bf16). The y-path keeps a bf16 xc so out_proj sees full
precision. in_proj/out_proj moving operands stay bf16: their outputs feed
the result directly and fp8 there fails the 2e-2 gate (sim: 2.3e-2).
Scales are folded into the conv weights / activation scale immediates.
Only the first D_INNER rows of w_in (x_and_res[..., :D_INNER]) and of
w_x (ssm_params[..., :D_INNER]) are ever used downstream, so the rest is
never computed.
"""

import sys

if '/opt/trn_rl_repo' not in sys.path:
    sys.path.insert(0, '/opt/trn_rl_repo')

import numpy as np
import ml_dtypes

D_MODEL, D_STATE, D_INNER = 1024, 16, 2048
EPS = 1e-5
B, S = 4, 4096
N_CORES = 8
T = (B * S) // N_CORES          # tokens per core
W = 512                         # token tile width
NT = T // W                     # token tiles per core
KI = D_MODEL // 128             # contraction tiles for in_proj
KC = D_INNER // 128             # contraction tiles for x_proj/out_proj
CT = D_INNER // 128             # channel tiles of d_inner
DT = D_MODEL // 128             # channel tiles of d_model

_BUILD_CACHE = {}



def _build(s_x: float, s_out: float):
    import concourse.tile as tile
    from concourse import bacc, mybir


    nc = bacc.Bacc("TRN2", target_bir_lowering=False, debug=False)
    f32 = mybir.dt.float32
    bf16 = mybir.dt.bfloat16
    fp8 = mybir.dt.float8e4
    AF = mybir.ActivationFunctionType
    ALU = mybir.AluOpType

    xT_d = nc.dram_tensor("xT", [D_MODEL, T], bf16, kind="ExternalInput")
    wi_d = nc.dram_tensor("wi", [D_MODEL, D_INNER], fp8, kind="ExternalInput")
    # wx packed pair-major on host: [128, KC*D_INNER], row p col k*D_INNER+c
    # = wxT[k*128+p, c]; device tile [128, KC, D_INNER] for DoubleRow pairs
    wx_d = nc.dram_tensor("wx_pk", [128, KC * D_INNER], fp8, kind="ExternalInput")
    wo_d = nc.dram_tensor("wo", [D_INNER, D_MODEL], fp8, kind="ExternalInput")
    # conv taps (s_in * w_conv) packed [128, CT*4]; fused conv bias [128, CT]
    wc_d = nc.dram_tensor("wc_v8", [128, CT * 4], f32, kind="ExternalInput")
    bc_d = nc.dram_tensor("bc", [128, CT], f32, kind="ExternalInput")
    bx_d = nc.dram_tensor("bx", [128, CT], f32, kind="ExternalInput")
    bo_d = nc.dram_tensor("bo", [128, DT], f32, kind="ExternalInput")
    # 4-token halo (only the last 3 are used by the conv): 4 bf16 = 8 bytes
    # per partition keeps the halo DMAs aligned and fully disjoint from the
    # psum-copy region of the xi tiles
    h0_d = nc.dram_tensor("h0", [128, CT * 4], bf16, kind="ExternalInput")
    out_d = nc.dram_tensor("out", [D_MODEL, T], bf16, kind="ExternalOutput")

    with tile.TileContext(nc) as tc:
        with (
            tc.tile_pool(name="weights", bufs=1) as wpool,
            tc.tile_pool(name="consts", bufs=1) as cpool,
            tc.tile_pool(name="xin", bufs=2) as xpool,
            tc.tile_pool(name="xi", bufs=2) as xipool,
            tc.tile_pool(name="acc", bufs=1) as accpool,
            tc.tile_pool(name="xc", bufs=1) as xcpool,
            tc.tile_pool(name="xc8", bufs=2) as xc8pool,
            tc.tile_pool(name="gate", bufs=3) as gatepool,
            tc.tile_pool(name="y", bufs=1) as ypool,
            tc.tile_pool(name="outp", bufs=1) as opool,
            tc.tile_pool(name="ps_in", bufs=3, space="PSUM") as ps_in,
            tc.tile_pool(name="ps_x", bufs=3, space="PSUM") as ps_x,
            tc.tile_pool(name="ps_o", bufs=2, space="PSUM") as ps_o,
        ):
            # ---- in_proj weights + constants first (gpsimd queue), so the
            # sync queue's first xT tiles aren't stuck behind 8 MB of weights
            wi_t = []
            for k in range(KI):
                t = wpool.tile([128, D_INNER], fp8, tag=f"wi{k}", name=f"wi{k}")
                nc.gpsimd.dma_start(t[:], wi_d[k * 128:(k + 1) * 128, :])
                wi_t.append(t)
            wc_t = cpool.tile([128, CT * 4], f32, name="wc_t")
            nc.gpsimd.dma_start(wc_t[:], wc_d[:, :])
            bc_t = cpool.tile([128, CT], f32, name="bc_t")
            nc.gpsimd.dma_start(bc_t[:], bc_d[:, :])
            bx_t = cpool.tile([128, CT], f32, name="bx_t")
            nc.gpsimd.dma_start(bx_t[:], bx_d[:, :])
            bo_t = cpool.tile([128, DT], f32, name="bo_t")
            nc.gpsimd.dma_start(bo_t[:], bo_d[:, :])

            # ---- PE warmup: ~5us of dummy matmuls during the initial DMA
            # wait so HAM un-throttles (1.2 -> 2.4 GHz) before real work
            warm = cpool.tile([128, 128], bf16, name="warm")
            nc.vector.memset(warm[:], 0.0)
            ps_w = ps_in.tile([128, 128], f32, tag="psin", name="ps_warm")
            for i in range(40):
                nc.tensor.matmul(ps_w[:], warm[:], warm[:],
                                 start=(i == 0), stop=(i == 39))

            # xi tiles hold [4-token halo | W tokens] of raw in_proj output,
            # bf16 so the conv taps run in the DVE 2x mode. The halo columns
            # are written by the PREVIOUS tile's a_group (SBUF->SBUF DMA),
            # or by h0 DMAs for j=0. Column 0 is never read.
            xi_tiles = {}

            def alloc_xi(j):
                if j in xi_tiles or j >= NT:
                    return
                xi_tiles[j] = [
                    xipool.tile([128, 4 + W], bf16, tag=f"xi{ct}",
                                name=f"xi{ct}_{j}", uniquify=True)
                    for ct in range(CT)]

            alloc_xi(0)
            for ct in range(CT):
                nc.gpsimd.dma_start(xi_tiles[0][ct][:, 0:4],
                                    h0_d[:, ct * 4:ct * 4 + 4])

            # first t-tile's activations on the sync queue, in parallel
            xt_tiles = {}
            for j in range(NT):
                if j > 0:
                    continue
                xt_tiles[j] = []
                for k in range(KI):
                    t = xpool.tile([128, W], bf16, tag=f"xt{k}", name=f"xt{k}_{j}")
                    nc.sync.dma_start(
                        t[:], xT_d[k * 128:(k + 1) * 128, j * W:(j + 1) * W])
                    xt_tiles[j].append(t)

            # remaining weights behind the first x tile
            wx_t = wpool.tile([128, KC, D_INNER], fp8, tag="wx", name="wx_t")
            for k in range(KC):
                nc.gpsimd.dma_start(
                    wx_t[:, k, :], wx_d[:, k * D_INNER:(k + 1) * D_INNER])
            wo_t = []
            for k in range(KC):
                t = wpool.tile([128, D_MODEL], fp8, tag=f"wo{k}", name=f"wo{k}")
                nc.gpsimd.dma_start(t[:], wo_d[k * 128:(k + 1) * 128, :])
                wo_t.append(t)

            xc_tiles = {}
            xc8_tiles = {}

            def load_xt(j):
                if j in xt_tiles or j >= NT:
                    return
                xt_tiles[j] = []
                for k in range(KI):
                    t = xpool.tile([128, W], bf16, tag=f"xt{k}", name=f"xt{k}_{j}")
                    nc.sync.dma_start(
                        t[:], xT_d[k * 128:(k + 1) * 128, j * W:(j + 1) * W])
                    xt_tiles[j].append(t)

            def a_group(j, ct):
                """in_proj + conv for one channel tile; silu deferred."""
                xt = xt_tiles[j]
                ps = ps_in.tile([128, W], f32, tag="psin", name=f"psin{ct}_{j}")
                for k in range(KI):
                    nc.tensor.matmul(
                        ps[:], wi_t[k][:, ct * 128:(ct + 1) * 128], xt[k][:],
                        start=(k == 0), stop=(k == KI - 1))
                xi = xi_tiles[j][ct]
                if j == 0:
                    # prologue: ScE is idle; freeing DVE here pulls the
                    # xc8(0) chain (which gates B(0)) earlier
                    nc.scalar.activation(xi[:, 4:4 + W], ps[:], AF.Copy)
                else:
                    nc.vector.tensor_copy(xi[:, 4:4 + W], ps[:])
                if j + 1 < NT:
                    # halo for the next token tile: SBUF->SBUF DMA, off DVE;
                    # last 4 tokens = cols [W, W+4)
                    nc.sync.dma_start(xi_tiles[j + 1][ct][:, 0:4],
                                      xi[:, W:W + 4])
                acc0 = accpool.tile([128, W], bf16, tag="accA", bufs=2,
                                    name=f"acc{ct}a_{j}")
                acc1 = accpool.tile([128, W], bf16, tag=f"acc{ct}b",
                                    name=f"acc{ct}b_{j}")
                ve = nc.vector
                ve.tensor_scalar_mul(
                    acc0[:], xi[:, 1:1 + W], wc_t[:, ct * 4:ct * 4 + 1])
                pp = [acc0, acc1]
                for k in range(1, 4):
                    src, dst = pp[(k - 1) % 2], pp[k % 2]
                    ve.scalar_tensor_tensor(
                        dst[:], xi[:, 1 + k:1 + k + W],
                        wc_t[:, ct * 4 + k:ct * 4 + k + 1],
                        src[:], op0=ALU.mult, op1=ALU.add)
                return pp[3 % 2]

            def a_silu(j, ct, acc):
                xct = xcpool.tile([128, W], bf16, tag=f"xc{ct}",
                                  name=f"xc{ct}_{j}")
                si = nc.scalar.activation(xct[:], acc[:], AF.Silu,
                                          bias=bc_t[:, ct:ct + 1], scale=1.0)
                si8 = nc.scalar.activation(xc8_tiles[j][:, ct, :], acc[:],
                                           AF.Silu, bias=bc_t[:, ct:ct + 1],
                                           scale=1.0)
                xc_tiles[j].append(xct)
                return si, si8

            def b_xproj_group(j, c2, y):
                xc = xc_tiles[j]
                xc8 = xc8_tiles[j]
                ps = ps_x.tile([128, W], f32, tag="psx", name=f"psx{c2}_{j}")
                for i in range(KC // 2):
                    nc.tensor.matmul(
                        ps[:], wx_t[:, 2 * i:2 * i + 2, c2 * 128:(c2 + 1) * 128],
                        xc8[:, 2 * i:2 * i + 2, :],
                        start=(i == 0), stop=(i == KC // 2 - 1),
                        perf_mode=mybir.MatmulPerfMode.DoubleRow)
                gate = gatepool.tile([128, W], bf16, tag="gate", name=f"g{c2}_{j}")
                sg = nc.scalar.activation(gate[:], ps[:], AF.Sigmoid,
                                          bias=bx_t[:, c2:c2 + 1], scale=s_x)
                sig_insts.append(sg)
                yt = ypool.tile([128, W], bf16, tag=f"y{c2}", name=f"y{c2}_{j}")
                nc.gpsimd.tensor_mul(yt[:], xc[c2][:], gate[:])
                y.append(yt)

            def b_outproj_group(j, dt, y):
                ps = ps_o.tile([128, W], f32, tag="pso", name=f"pso{dt}_{j}")
                for k in range(KC):
                    nc.tensor.matmul(
                        ps[:], wo_t[k][:, dt * 128:(dt + 1) * 128], y[k][:],
                        start=(k == 0), stop=(k == KC - 1))
                ot = opool.tile([128, W], bf16, tag=f"ot{dt}", name=f"ot{dt}_{j}")
                # identity is in every ACT table set: no table-switch cost,
                # and ScE has the fast PSUM port
                nc.scalar.activation(ot[:], ps[:], AF.Identity,
                                     bias=bo_t[:, dt:dt + 1], scale=s_out)
                nc.sync.dma_start(
                    out_d[dt * 128:(dt + 1) * 128, j * W:(j + 1) * W], ot[:])

            def emit_a_phase(j):
                """Emit all of in_proj+conv+silu for tile j (pipeline fill)."""
                xc_tiles[j] = []
                xc8_tiles[j] = xc8pool.tile([128, CT, W], fp8, tag="xc8",
                                            name=f"xc8_{j}")
                accs = [a_group(j, ct) for ct in range(CT)]
                for ct in range(CT):
                    a_silu(j, ct, accs[ct])

            # ---- software pipeline: B(j) interleaved with A(j+1) ----
            from concourse.tile import add_dep_helper
            sig_insts = []
            load_xt(0)
            load_xt(1)
            alloc_xi(1)
            emit_a_phase(0)
            for j in range(NT):
                nxt = j + 1
                load_xt(nxt + 1)
                alloc_xi(nxt + 1)
                if nxt < NT:
                    xc_tiles[nxt] = []
                    xc8_tiles[nxt] = xc8pool.tile([128, CT, W], fp8, tag="xc8",
                                                  name=f"xc8_{nxt}")
                a_accs = []
                y = []
                # 16 x_proj groups interleaved with 16 A-groups of next tile
                for c2 in range(CT):
                    b_xproj_group(j, c2, y)
                    if nxt < NT:
                        a_accs.append((c2, a_group(nxt, c2)))
                for dt in range(DT):
                    b_outproj_group(j, dt, y)
                if nxt < NT:
                    prev_act = sig_insts[-1] if sig_insts else None
                    for ct, acc in a_accs:
                        for si in a_silu(nxt, ct, acc):
                            if prev_act is not None:
                                add_dep_helper(si.ins, prev_act.ins, sync=False,
                                               reason="group ACT funcs to avoid table thrash")
                            prev_act = si

    nc.compile()
    return nc


def _quantize(w):
    s = np.float32(max(np.abs(w).mean(dtype=np.float64), EPS))
    return np.clip(np.round(w / s), -1.0, 1.0).astype(np.float32), s


def kernel(x, w_in, b_in, w_conv, b_conv, w_x, b_x, w_out, b_out,
           _trace=False, _trace_kwargs=None):
    from concourse import bass_utils

    x = np.asarray(x, dtype=np.float32)
    w_in = np.asarray(w_in, dtype=np.float32)
    b_in = np.asarray(b_in, dtype=np.float32)
    w_conv = np.asarray(w_conv, dtype=np.float32)
    b_conv = np.asarray(b_conv, dtype=np.float32)
    w_x = np.asarray(w_x, dtype=np.float32)
    b_x = np.asarray(b_x, dtype=np.float32)
    w_out = np.asarray(w_out, dtype=np.float32)
    b_out = np.asarray(b_out, dtype=np.float32)

    # ---- host-side BitNet quantization (exact ternary) ----
    wq_in, s_in = _quantize(w_in)     # [2*D_INNER, D_MODEL]
    wq_x, s_x = _quantize(w_x)        # [D_STATE+D_MODEL+D_INNER, D_INNER]
    wq_out, s_out = _quantize(w_out)  # [D_MODEL, D_INNER]
    wq_in = wq_in[:D_INNER]           # res half unused downstream
    wq_x_d = wq_x[:D_INNER]           # only delta rows used

    fp8 = ml_dtypes.float8_e4m3
    wiT = np.ascontiguousarray(wq_in.T).astype(fp8)       # [D_MODEL, D_INNER]
    wxT = np.ascontiguousarray(wq_x_d.T).astype(fp8)      # [D_INNER, D_INNER]
    woT = np.ascontiguousarray(wq_out.T).astype(fp8)      # [D_INNER, D_MODEL]
    # pair-major for DoubleRow: [128, KC*D_INNER], row p col k*D_INNER+c
    wx_pk = np.ascontiguousarray(
        wxT.reshape(KC, 128, D_INNER).transpose(1, 0, 2)
        .reshape(128, KC * D_INNER))

    # conv taps with in_proj scale folded in; bias absorbs b_in through the taps
    wc = (s_in * w_conv[:, 0, :]).astype(np.float32)             # [D_INNER, 4]
    bc = (b_in[:D_INNER] * w_conv[:, 0, :].sum(axis=1)
          + b_conv).astype(np.float32)                           # [D_INNER]
    bx = b_x[:D_INNER].astype(np.float32)
    bo = b_out.astype(np.float32)

    # pack per-channel constants as [128, CT] (partition = channel % 128)
    def pack(v, ncols):
        return np.ascontiguousarray(
            v.reshape(ncols, 128).T.astype(np.float32))  # [128, ncols]

    wc_pk = np.ascontiguousarray(
        wc.reshape(CT, 128, 4).transpose(1, 0, 2).reshape(128, CT * 4))
    bc_pk = pack(bc, CT)
    bx_pk = pack(bx, CT)
    bo_pk = pack(bo, DT)

    # ---- shard inputs ----
    x_flat = x.reshape(B * S, D_MODEL)
    xT = np.ascontiguousarray(x_flat.T)                   # [D_MODEL, B*S] f32
    xT_bf = xT.astype(ml_dtypes.bfloat16)

    # raw in_proj value that makes x_inner == 0 (sequence-start padding)
    pad_raw = (-b_in[:D_INNER] / s_in).astype(np.float32)

    in_maps = []
    for c in range(N_CORES):
        t0 = c * T
        if t0 % S == 0:
            h0 = np.repeat(pad_raw[:, None], 4, axis=1)   # [D_INNER, 4]
        else:
            h0 = wq_in @ x_flat[t0 - 4:t0].T              # [D_INNER, 4]
        h0_pk = np.ascontiguousarray(
            h0.reshape(CT, 128, 4).transpose(1, 0, 2).reshape(128, CT * 4)
        ).astype(ml_dtypes.bfloat16)
        in_maps.append({
            "xT": np.ascontiguousarray(xT_bf[:, t0:t0 + T]),
            "wi": wiT, "wx_pk": wx_pk, "wo": woT,
            "wc_v8": wc_pk, "bc": bc_pk, "bx": bx_pk, "bo": bo_pk,
            "h0": h0_pk,
        })

    key = (float(s_x), float(s_out))
    if key not in _BUILD_CACHE:
        _BUILD_CACHE[key] = _build(float(s_x), float(s_out))
    nc = _BUILD_CACHE[key]

    kwargs = {}
    if _trace:
        kwargs["trace"] = True
        if _trace_kwargs:
            kwargs.update(_trace_kwargs)
    res = bass_utils.run_bass_kernel_spmd(
        nc, in_maps, core_ids=list(range(N_CORES)), **kwargs)
    kernel.last_results = res

    full = np.concatenate([res.results[c]["out"] for c in range(N_CORES)],
                          axis=1)                          # [D_MODEL, B*S]
    return np.ascontiguousarray(full.T).reshape(B, S, D_MODEL).astype(np.float32)

